# revision 10
# baseline (speedup 1.0000x reference)
"""Bidirectional 2-layer LSTM -> dense, Trainium2 Bass kernel.

Key insight: reference does outputs_btd[-1] where outputs_btd is [B, T, 2H],
so the result depends ONLY on batch row 255. We compute just that row's
forward and backward LSTM chains (one NeuronCore each), with all state kept
in a transposed "gates-on-partitions" layout so the per-step elementwise ops
are [128, small] instead of [1, 1024].

Layouts (per direction core):
  - vectors v[0:256] are stored as [128, 2] tiles: col h holds v[128h:128h+128]
  - gate pre-activations z[0:1024] as [128, 8]: col g holds z[128g:128(g+1)]
  - gates permuted host-side from TF order (i,j,f,o) to (i,f,o,j) so sigmoid
    covers cols 0:6 and tanh cols 6:8; FORGET_BIAS folded into the f bias.
  - hidden history HS [128, 2T]: cols (2t, 2t+1) = h_t halves -> directly
    usable as matmul rhs [128,1] columns and as strided lhsT for the dense.
  - input projections P = x @ Wx + b precomputed on device, stored t-major
    [128, 8T] so the hot loop reads a contiguous [128, 8] slice per step.
All [128, .] inputs are packed into one DRAM tensor (and [1, .] into another)
so compute instructions only ever wait on a single DMA-queue semaphore
(hardware encodes very few sync waits per instruction).
"""

import numpy as np

H = 256
T = 512
D = 128
OUT = 128
FORGET_BIAS = 1.0

# TF gate order i,j,f,o -> reorder columns to i,f,o,j
_PERM = np.r_[0:256, 512:768, 768:1024, 256:512]

# big-tensor column offsets
_OFF = {}
_c = 0
for _name, _w in [("w0x", 1024), ("w0ha", 1024), ("w0hb", 1024),
                  ("w1xa", 1024), ("w1xb", 1024), ("w1ha", 1024),
                  ("w1hb", 1024), ("xT", T), ("st", 8),
                  ("wda", OUT), ("wdb", OUT)]:
    _OFF[_name] = (_c, _c + _w)
    _c += _w
_BIGW = _c


def _build_program():
    import concourse.bass as bass
    import concourse.mybir as mybir
    from concourse import bacc, tile

    fp32 = mybir.dt.float32
    nc = bacc.Bacc(None, target_bir_lowering=False)

    big_d = nc.declare_dram_parameter("big", [128, _BIGW], fp32, isOutput=False)
    brow_d = nc.declare_dram_parameter("brow", [1, 2560], fp32, isOutput=False)
    out_d = nc.declare_dram_parameter("out", [T, OUT], fp32, isOutput=True)

    SIG = mybir.ActivationFunctionType.Sigmoid
    TANH = mybir.ActivationFunctionType.Tanh

    with tile.TileContext(nc) as tc:
        with (
            tc.tile_pool(name="pool", bufs=1) as pool,
            tc.tile_pool(name="psum", bufs=1, space="PSUM") as psum,
        ):
            big = pool.tile([128, _BIGW], fp32, tag="big")
            brow = pool.tile([1, 2560], fp32, tag="brow")
            P0 = pool.tile([128, 8 * T], fp32, tag="P0")   # t-major input proj L0
            P1 = pool.tile([128, 8 * T], fp32, tag="P1")
            HS0 = pool.tile([128, 2 * T], fp32, tag="HS0")
            HS1 = pool.tile([128, 2 * T], fp32, tag="HS1")
            zsb = pool.tile([128, 8], fp32, tag="zsb")
            G = pool.tile([128, 8], fp32, tag="G")
            tmp = pool.tile([128, 2], fp32, tag="tmp")
            t2 = pool.tile([128, 2], fp32, tag="t2")
            ca = pool.tile([128, 2], fp32, tag="ca")
            cb = pool.tile([128, 2], fp32, tag="cb")
            tch = pool.tile([128, 2], fp32, tag="tch")
            outsb = pool.tile([128, OUT], fp32, tag="outsb")

            zmm = psum.tile([128, 8], fp32, tag="zmm")
            pp = psum.tile([128, T], fp32, tag="pp")
            psd = psum.tile([128, OUT], fp32, tag="psd")

            nc.sync.dma_start(big[:], big_d[:])
            nc.sync.dma_start(brow[:], brow_d[:])
            tc.strict_bb_all_engine_barrier()

            def bigs(name):
                a, b = _OFF[name]
                return big[:, a:b]

            w0x, w0ha, w0hb = bigs("w0x"), bigs("w0ha"), bigs("w0hb")
            w1xa, w1xb = bigs("w1xa"), bigs("w1xb")
            w1ha, w1hb = bigs("w1ha"), bigs("w1hb")
            xT, st = bigs("xT"), bigs("st")
            wda, wdb = bigs("wda"), bigs("wdb")
            b0, b1 = brow[:, 0:1024], brow[:, 1024:2048]
            ones = brow[:, 2048:2560]

            # t-major strided views of the projection buffers: [128, T, 8]
            P0v = P0[:].rearrange("p (t g) -> p t g", g=8)
            P1v = P1[:].rearrange("p (t g) -> p t g", g=8)

            def precompute(Pv, wxs, brow_, rhss):
                # P[:, t, g] = (sum_parts wx.T @ rhs + b)[:, t]
                for g in range(8):
                    gs = slice(128 * g, 128 * (g + 1))
                    for i, (wx, rhs) in enumerate(zip(wxs, rhss)):
                        nc.tensor.matmul(pp[:], wx[:, gs], rhs,
                                         start=(i == 0), stop=False)
                    nc.tensor.matmul(pp[:], brow_[:, gs], ones,
                                     start=False, stop=True)
                    nc.vector.tensor_copy(Pv[:, :, g], pp[:])

            def lstm_phase(Pbuf, wha, whb, HS, c_init, h_init_a, h_init_b):
                c_prev = c_init
                for t in range(T):
                    if t == 0:
                        ra, rb = h_init_a, h_init_b
                    else:
                        ra = HS[:, 2 * t - 2:2 * t - 1]
                        rb = HS[:, 2 * t - 1:2 * t]
                    for g in range(8):
                        gs = slice(128 * g, 128 * (g + 1))
                        nc.tensor.matmul(zmm[:, g:g + 1], wha[:, gs], ra,
                                         start=True, stop=False)
                        nc.tensor.matmul(zmm[:, g:g + 1], whb[:, gs], rb,
                                         start=False, stop=True)
                    nc.vector.tensor_add(zsb[:], zmm[:], Pbuf[:, 8 * t:8 * t + 8])
                    nc.scalar.activation(G[:, 0:6], zsb[:, 0:6], SIG)
                    nc.scalar.activation(G[:, 6:8], zsb[:, 6:8], TANH)
                    c_cur = ca if (t % 2 == 0) else cb
                    nc.vector.tensor_mul(tmp[:], G[:, 0:2], G[:, 6:8])
                    nc.vector.tensor_mul(t2[:], G[:, 2:4], c_prev)
                    nc.vector.tensor_add(c_cur[:], t2[:], tmp[:])
                    nc.scalar.activation(tch[:], c_cur[:], TANH)
                    nc.vector.tensor_mul(HS[:, 2 * t:2 * t + 2], G[:, 4:6], tch[:])
                    c_prev = c_cur[:]

            # ---- layer 0 ----
            precompute(P0v, [w0x], b0, [xT])
            lstm_phase(P0, w0ha, w0hb, HS0,
                       st[:, 0:2], st[:, 2:3], st[:, 3:4])

            # ---- layer 1 (input = layer0 hidden history) ----
            HS0v = HS0[:].rearrange("p (t h) -> p t h", h=2)
            precompute(P1v, [w1xa, w1xb], b1, [HS0v[:, :, 0], HS0v[:, :, 1]])
            lstm_phase(P1, w1ha, w1hb, HS1,
                       st[:, 4:6], st[:, 6:7], st[:, 7:8])

            # ---- dense: out[t, :] = hs1[t] @ Wd_half ----
            HS1v = HS1[:].rearrange("p (t h) -> p t h", h=2)
            for m in range(4):
                ts_ = slice(128 * m, 128 * (m + 1))
                nc.tensor.matmul(psd[:], HS1v[:, ts_, 0], wda,
                                 start=True, stop=False)
                nc.tensor.matmul(psd[:], HS1v[:, ts_, 1], wdb,
                                 start=False, stop=True)
                nc.vector.tensor_copy(outsb[:], psd[:])
                nc.sync.dma_start(out_d[ts_, :], outsb[:])

    nc.compile()
    return nc


def _direction_inputs(xr, state, W0, b0, W1, b1, Wd_half):
    """Host-side tensor prep for one direction (xr already time-ordered
    for this direction's scan)."""
    W0p = np.ascontiguousarray(W0[:, _PERM], np.float32)
    W1p = np.ascontiguousarray(W1[:512, _PERM], np.float32)
    b0p = b0[_PERM].astype(np.float32).copy()
    b1p = b1[_PERM].astype(np.float32).copy()
    b0p[256:512] += FORGET_BIAS
    b1p[256:512] += FORGET_BIAS

    def halves(v):  # [256] -> [128, 2]
        return np.stack([v[:128], v[128:]], axis=1).astype(np.float32)

    c0, h0 = state[0:256], state[256:512]
    c1, h1 = state[512:768], state[768:1024]
    st = np.concatenate([halves(c0), halves(h0), halves(c1), halves(h1)],
                        axis=1)  # [128, 8]

    parts = {
        "w0x": W0p[0:128],
        "w0ha": W0p[128:256],
        "w0hb": W0p[256:384],
        "w1xa": W1p[0:128],
        "w1xb": W1p[128:256],
        "w1ha": W1p[256:384],
        "w1hb": W1p[384:512],
        "xT": xr.T.astype(np.float32),
        "st": st,
        "wda": Wd_half[0:128].astype(np.float32),
        "wdb": Wd_half[128:256].astype(np.float32),
    }
    big = np.zeros((128, _BIGW), np.float32)
    for k, (a, b) in _OFF.items():
        big[:, a:b] = parts[k]
    brow = np.zeros((1, 2560), np.float32)
    brow[0, 0:1024] = b0p
    brow[0, 1024:2048] = b1p
    brow[0, 2048:2560] = 1.0
    return {"big": big, "brow": brow}


_CACHE = {}


def kernel(x, fw_state, bw_state, Wf0, bf0, Wf1, bf1, Wb0, bb0, Wb1, bb1,
           Wd, bd):
    from concourse.bass_utils import run_bass_kernel_spmd

    x = np.asarray(x, np.float32)
    xr = x[-1]  # [T, D] -- the only batch row the reference output uses

    fw_in = _direction_inputs(xr, np.asarray(fw_state, np.float32)[-1],
                              np.asarray(Wf0), np.asarray(bf0),
                              np.asarray(Wf1), np.asarray(bf1),
                              np.asarray(Wd)[0:256])
    bw_in = _direction_inputs(xr[::-1], np.asarray(bw_state, np.float32)[-1],
                              np.asarray(Wb0), np.asarray(bb0),
                              np.asarray(Wb1), np.asarray(bb1),
                              np.asarray(Wd)[256:512])

    if "nc" not in _CACHE:
        _CACHE["nc"] = _build_program()
    nc = _CACHE["nc"]

    core_ids = list(range(8))
    in_maps = [fw_in, bw_in] + [fw_in] * 6
    res = run_bass_kernel_spmd(nc, in_maps, core_ids)
    _CACHE["last_result"] = res
    out_fw = np.asarray(res.results[0]["out"])
    out_bw = np.asarray(res.results[1]["out"])

    logits = out_fw + out_bw[::-1] + np.asarray(bd, np.float32)[None, :]
    return logits.astype(np.float32)
